# revision 1
# baseline (speedup 1.0000x reference)
"""PET tube-of-response backprojection on 8 TRN2 NeuronCores.

Strategy: slice-sharding. Every LOR crosses every slice of the dominant axis,
so giving core c slices [16c, 16c+16) of all three backprojections is
perfectly balanced, needs no collective, and each core's output is disjoint.

Per (axis, 128-LOR chunk, slice) the scatter is computed as a dense separable
outer product on the tensor engine:
  CL  = clamp(iota, ix0-1, ix0+1)            (DVE, per-partition window bounds)
  X   = (1+K)*iota - K*CL                    (DVE; == iota in-window, huge outside)
  SQ  = Square(sqrt(c)*X - sqrt(c)*u)        (ACT; c*(i-u)^2, huge outside)
  W   = Exp(-SQ [+ ln proj])                 (ACT; Gaussian weight, 0 outside)
  PSUM[k] += Wx^T @ Wy                       (PE, fp32 accumulation over chunks)

The voxel-index decision ix0 = round((cx+100)/1.5625 - 0.5) reproduces the
fp32 reference bit-exactly: cx via mult+add, the division via a
multiply + exact-residual correction (q = y*0.64; r = ((y-q)-0.5q)-0.0625q;
u' = q + r*0.64), and round-to-nearest-even via the +-1.5*2^23 magic add.
"""

import math
import sys

sys.path.insert(0, "/opt/trn_rl_repo")
sys.path.insert(0, "/opt/trn_rl_repo/concourse")

import numpy as np

V = 1.5625
INV_V = float(np.float32(0.64))
NEG_O = 100.0
SIGMA2 = 9.0 * math.pi / 4.0
C = 0.5 * V * V / SIGMA2
SQRT_C = math.sqrt(C)
MAGIC = 12582912.0
KCL = 1024.0

N_CORES = 8
N_K = 16          # slices per core
N_CHUNKS = 128    # 128-LOR chunks
N_LORS = N_CHUNKS * 128

ROTATIONS = {"x": [1, 2, 0], "y": [0, 2, 1], "z": [0, 1, 2]}
BACK_ROTATIONS_IMAGE = {"x": [1, 2, 0], "y": [1, 0, 2], "z": [0, 1, 2]}
AXES = ("x", "y", "z")

_CACHE = {}


def _build_kernel(repeat=1):
    from concourse import mybir, tile, bacc

    DT = mybir.dt
    F32 = DT.float32
    BF16 = DT.bfloat16
    AO = mybir.AluOpType
    AF = mybir.ActivationFunctionType
    n_chunks, n_k, n_axes = N_CHUNKS, N_K, 3

    nc = bacc.Bacc("TRN2", target_bir_lowering=False, debug=False)
    lors_d = [nc.dram_tensor(f"lors{a}", [4, N_LORS], F32, kind="ExternalInput")
              for a in range(n_axes)]
    proj_d = [nc.dram_tensor(f"proj{a}", [N_LORS], F32, kind="ExternalInput")
              for a in range(n_axes)]
    iota_d = nc.dram_tensor("iota", [128, 128], F32, kind="ExternalInput")
    tval_d = nc.dram_tensor("tvals", [128, n_k], F32, kind="ExternalInput")
    slab_d = [nc.dram_tensor(f"slab{a}", [128, n_k, 128], F32,
                             kind="ExternalOutput") for a in range(n_axes)]

    with tile.TileContext(nc) as tc:
        with (
            tc.tile_pool(name="const", bufs=1) as constp,
            tc.tile_pool(name="pre", bufs=1) as prep,
            tc.tile_pool(name="work", bufs=4) as workp,
            tc.tile_pool(name="out", bufs=2) as outp,
            tc.tile_pool(name="ps", bufs=2, space="PSUM") as psp,
        ):
            IOTA = constp.tile([128, 128], F32, tag="iota")
            nc.sync.dma_start(IOTA[:], iota_d[:])
            JT = constp.tile([128, 128], F32, tag="jt")
            nc.vector.tensor_scalar(JT[:], IOTA[:], KCL + 1.0, None, op0=AO.mult)
            TT = constp.tile([128, n_k], F32, tag="tt")
            nc.sync.dma_start(TT[:], tval_d[:])

            rep_ctx = tc.For_i(0, repeat, 1) if repeat > 1 else None
            if rep_ctx is not None:
                rep_ctx.__enter__()
            for a in range(n_axes):
                comp = []
                for r in range(4):
                    t_ = prep.tile([128, n_chunks], F32, tag=f"comp{r}")
                    nc.sync.dma_start(
                        t_[:], lors_d[a][r, :].rearrange("(p c) -> p c", p=128))
                    comp.append(t_)
                P1X, P1Y, P2X, P2Y = comp
                PRJ = prep.tile([128, n_chunks], F32, tag="prj")
                nc.sync.dma_start(PRJ[:],
                                  proj_d[a][:].rearrange("(p c) -> p c", p=128))
                LNP = prep.tile([128, n_chunks], F32, tag="lnp")
                nc.scalar.activation(LNP[:], PRJ[:], AF.Ln)

                sides = []
                for (P1, P2, nm) in ((P1X, P2X, "x"), (P1Y, P2Y, "y")):
                    DX = prep.tile([128, n_chunks], F32, tag="dxt")
                    nc.vector.tensor_tensor(DX[:], P2[:], P1[:], op=AO.subtract)
                    CX = prep.tile([128, n_chunks, n_k], F32, tag="chainA")
                    tb = TT[:].unsqueeze(1).broadcast_to([128, n_chunks, n_k])
                    dxb = DX[:].unsqueeze(2).broadcast_to([128, n_chunks, n_k])
                    p1b = P1[:].unsqueeze(2).broadcast_to([128, n_chunks, n_k])
                    nc.vector.tensor_tensor(CX[:], tb, dxb, op=AO.mult)
                    nc.vector.tensor_tensor(CX[:], CX[:], p1b, op=AO.add)
                    Y_ = prep.tile([128, n_chunks, n_k], F32, tag="chainC")
                    nc.vector.tensor_scalar(Y_[:], CX[:], NEG_O, None, op0=AO.add)
                    Q_ = prep.tile([128, n_chunks, n_k], F32, tag="chainD")
                    nc.vector.tensor_scalar(Q_[:], Y_[:], INV_V, None, op0=AO.mult)
                    R_ = prep.tile([128, n_chunks, n_k], F32, tag="chainA")
                    nc.vector.tensor_tensor(R_[:], Y_[:], Q_[:], op=AO.subtract)
                    nc.vector.scalar_tensor_tensor(R_[:], Q_[:], -0.5, R_[:],
                                                   op0=AO.mult, op1=AO.add)
                    nc.vector.scalar_tensor_tensor(R_[:], Q_[:], -0.0625, R_[:],
                                                   op0=AO.mult, op1=AO.add)
                    U = prep.tile([128, n_chunks, n_k], F32, tag="chainB")
                    nc.vector.scalar_tensor_tensor(U[:], R_[:], INV_V, Q_[:],
                                                   op0=AO.mult, op1=AO.add)
                    nc.vector.tensor_scalar(U[:], U[:], 0.5, None, op0=AO.subtract)
                    IX0 = prep.tile([128, n_chunks, n_k], F32, tag="chainA")
                    nc.vector.tensor_scalar(IX0[:], U[:], MAGIC, MAGIC,
                                            op0=AO.add, op1=AO.subtract)
                    LO = prep.tile([128, n_chunks, n_k], F32, tag=f"lo{nm}")
                    nc.vector.tensor_scalar(LO[:], IX0[:], 1.0, None,
                                            op0=AO.subtract)
                    EN = prep.tile([128, n_chunks, n_k], F32, tag=f"en{nm}")
                    nc.vector.tensor_scalar(EN[:], IX0[:], 1.0, None, op0=AO.add)
                    BQ = prep.tile([128, n_chunks, n_k], F32, tag=f"bq{nm}")
                    nc.vector.tensor_scalar(BQ[:], U[:], -SQRT_C, None, op0=AO.mult)
                    sides.append((LO, EN, BQ))
                (LOX, ENX, BQX), (LOY, ENY, BQY) = sides

                PS = psp.tile([128, n_k, 128], F32, tag="ps")
                bank_slices = min(n_k, 4)

                for c in range(n_chunks):
                    first, last = c == 0, c == n_chunks - 1
                    for k in range(n_k):
                        tiles = []
                        for (LO, EN, BQ, nm) in ((LOX, ENX, BQX, "x"),
                                                 (LOY, ENY, BQY, "y")):
                            CL = workp.tile([128, 128], F32, tag=f"cl{nm}")
                            nc.vector.tensor_scalar(
                                CL[:], IOTA[:], LO[:, c, k:k + 1],
                                EN[:, c, k:k + 1], op0=AO.max, op1=AO.min)
                            MI = workp.tile([128, 128], F32, tag=f"mi{nm}")
                            nc.vector.scalar_tensor_tensor(
                                MI[:], CL[:], -KCL, JT[:], op0=AO.mult, op1=AO.add)
                            SQ = workp.tile([128, 128], F32, tag=f"sq{nm}")
                            nc.scalar.activation(SQ[:], MI[:], AF.Square,
                                                 bias=BQ[:, c, k:k + 1],
                                                 scale=SQRT_C)
                            W = workp.tile([128, 128], BF16, tag=f"w{nm}")
                            if nm == "y":
                                nc.scalar.activation(W[:], SQ[:], AF.Exp,
                                                     bias=LNP[:, c:c + 1],
                                                     scale=-1.0)
                            else:
                                nc.scalar.activation(W[:], SQ[:], AF.Exp,
                                                     scale=-1.0)
                            tiles.append(W)
                        nc.tensor.matmul(PS[:, k, :], tiles[0][:], tiles[1][:],
                                         start=first and (k % bank_slices == 0),
                                         stop=last and
                                         (k % bank_slices == bank_slices - 1))

                OUT = outp.tile([128, n_k, 128], F32, tag="out")
                nc.vector.tensor_copy(OUT[:], PS[:])
                nc.sync.dma_start(slab_d[a][:], OUT[:])
            if rep_ctx is not None:
                rep_ctx.__exit__(None, None, None)

    nc.finalize()
    return nc


def _host_tvals():
    zc = np.float32(-100.0) + (np.arange(128, dtype=np.float32)
                               + np.float32(0.5)) * np.float32(1.5625)
    return (zc + np.float32(100.0)) / np.float32(200.0)


def _host_prepare(inputs):
    iota = np.broadcast_to(np.arange(128, dtype=np.float32), (128, 128)).copy()
    t_all = _host_tvals()
    lors = {"x": inputs["xlors"], "y": inputs["ylors"], "z": inputs["zlors"]}
    proj = {"x": inputs["xproj"], "y": inputs["yproj"], "z": inputs["zproj"]}
    base = {}
    for ai, a in enumerate(AXES):
        cols = ROTATIONS[a] + [i + 3 for i in ROTATIONS[a]]
        l = np.asarray(lors[a]).astype(np.float32)[:, cols]
        base[f"lors{ai}"] = np.ascontiguousarray(
            np.stack([l[:, 0], l[:, 1], l[:, 3], l[:, 4]]))
        base[f"proj{ai}"] = np.ascontiguousarray(
            np.asarray(proj[a]), dtype=np.float32)
    in_maps = []
    for cid in range(N_CORES):
        m = dict(base)
        m["iota"] = iota
        tk = t_all[cid * N_K:(cid + 1) * N_K]
        m["tvals"] = np.broadcast_to(tk, (128, N_K)).copy()
        in_maps.append(m)
    return in_maps


def _host_gather(results):
    outs = []
    for ai, a in enumerate(AXES):
        bp = np.concatenate(
            [np.transpose(r[f"slab{ai}"], (0, 2, 1)) for r in results], axis=2)
        outs.append(np.ascontiguousarray(
            np.transpose(bp, BACK_ROTATIONS_IMAGE[a]).astype(np.float32)))
    return tuple(outs)


def kernel(image, xlors, ylors, zlors, xproj, yproj, zproj):
    from concourse.bass_utils import run_bass_kernel_spmd

    if "nc" not in _CACHE:
        _CACHE["nc"] = _build_kernel()
    nc = _CACHE["nc"]
    inputs = dict(xlors=np.asarray(xlors), ylors=np.asarray(ylors),
                  zlors=np.asarray(zlors), xproj=np.asarray(xproj),
                  yproj=np.asarray(yproj), zproj=np.asarray(zproj))
    in_maps = _host_prepare(inputs)
    res = run_bass_kernel_spmd(nc, in_maps, core_ids=list(range(N_CORES)))
    return _host_gather(res.results)



# revision 4
# speedup vs baseline: 1.4615x; 1.4615x over previous
"""PET tube-of-response backprojection on 8 TRN2 NeuronCores.

Strategy: slice-sharding. Every LOR crosses every slice of the dominant axis,
so giving core c slices [16c, 16c+16) of all three backprojections is
perfectly balanced, needs no collective, and each core's output is disjoint.

Per (axis, 128-LOR chunk, slice) the scatter is computed as a dense separable
outer product on the tensor engine:
  CL16 = clamp(-32*iota, -32*(ix0+1), -32*(ix0-1))   (DVE ts, bf16 4x mode;
         -32*i is exact in bf16 for i<128: 7-bit int times 2^5)
  MI   = (CL16 - u) + 33*iota                        (DVE stt, per-partition
         scalar u; == iota-u in-window, |MI|>=30 outside)
  SQ   = Square(sqrt(c)*MI)    over [128, 2*16*128]  (ACT, k-batched)
  W    = Exp(-SQ [+ ln proj])  over [128, 16*128]    (ACT, k-batched, bf16)
  PSUM[k] += Wx^T @ Wy                               (PE, fp32 accumulation)

vs. the previous version this batches the ACT engine over all 16 slices
(N=2048+ per instruction instead of N=128), eliminating ~23000 ACT
instructions x ~290ns fixed overhead that made the kernel ACT-bound.

The voxel-index decision ix0 = round((cx+100)/1.5625 - 0.5) reproduces the
fp32 reference bit-exactly: cx via mult+add, the division via a
multiply + exact-residual correction (q = y*0.64; r = ((y-q)-0.5q)-0.0625q;
u' = q + r*0.64), and round-to-nearest-even via the +-1.5*2^23 magic add.
The Gaussian argument (i-u) is evaluated with ~2.4e-4 abs error (two fp32
roundings at magnitude ~4e3), giving ~1e-4 relative weight error.
"""

import math
import sys

sys.path.insert(0, "/opt/trn_rl_repo")
sys.path.insert(0, "/opt/trn_rl_repo/concourse")

import numpy as np

V = 1.5625
INV_V = float(np.float32(0.64))
NEG_O = 100.0
SIGMA2 = 9.0 * math.pi / 4.0
C = 0.5 * V * V / SIGMA2
SQRT_C = math.sqrt(C)
MAGIC = 12582912.0
KCL = 32.0  # window-marker scale: out-of-window |MI| >= 32-1.5, SQ >= 160

N_CORES = 8
N_K = 16          # slices per core
N_CHUNKS = 128    # 128-LOR chunks
N_LORS = N_CHUNKS * 128

ROTATIONS = {"x": [1, 2, 0], "y": [0, 2, 1], "z": [0, 1, 2]}
BACK_ROTATIONS_IMAGE = {"x": [1, 2, 0], "y": [1, 0, 2], "z": [0, 1, 2]}
AXES = ("x", "y", "z")

_CACHE = {}


def _build_kernel(repeat=1):
    from concourse import mybir, tile, bacc

    DT = mybir.dt
    F32 = DT.float32
    BF16 = DT.bfloat16
    AO = mybir.AluOpType
    AF = mybir.ActivationFunctionType
    n_chunks, n_k, n_axes = N_CHUNKS, N_K, 3

    nc = bacc.Bacc("TRN2", target_bir_lowering=False, debug=False)
    lors_d = [nc.dram_tensor(f"lors{a}", [4, N_LORS], F32, kind="ExternalInput")
              for a in range(n_axes)]
    proj_d = [nc.dram_tensor(f"proj{a}", [N_LORS], F32, kind="ExternalInput")
              for a in range(n_axes)]
    iota_d = nc.dram_tensor("iota", [128, 128], F32, kind="ExternalInput")
    tval_d = nc.dram_tensor("tvals", [128, n_k], F32, kind="ExternalInput")
    slab_d = [nc.dram_tensor(f"slab{a}", [128, n_k, 128], F32,
                             kind="ExternalOutput") for a in range(n_axes)]

    with tile.TileContext(nc) as tc:
        with (
            tc.tile_pool(name="const", bufs=1) as constp,
            tc.tile_pool(name="pre", bufs=1) as prep,
            tc.tile_pool(name="work", bufs=4) as workp,
            tc.tile_pool(name="sq", bufs=2) as sqp,
            tc.tile_pool(name="w", bufs=2) as wp,
            tc.tile_pool(name="out", bufs=2) as outp,
            tc.tile_pool(name="ps", bufs=2, space="PSUM") as psp,
        ):
            IOTA = constp.tile([128, 128], F32, tag="iota")
            nc.sync.dma_start(IOTA[:], iota_d[:])
            # JT33 = 33*iota (fp32); ION32 = -32*iota (bf16, exact)
            JT33 = constp.tile([128, 128], F32, tag="jt33")
            nc.vector.tensor_scalar(JT33[:], IOTA[:], KCL + 1.0, None,
                                    op0=AO.mult)
            ION32 = constp.tile([128, 128], BF16, tag="ion32")
            nc.vector.tensor_scalar(ION32[:], IOTA[:], -KCL, None, op0=AO.mult)
            TT = constp.tile([128, n_k], F32, tag="tt")
            nc.sync.dma_start(TT[:], tval_d[:])

            rep_ctx = tc.For_i(0, repeat, 1) if repeat > 1 else None
            if rep_ctx is not None:
                rep_ctx.__enter__()
            for a in range(n_axes):
                comp = []
                for r in range(4):
                    t_ = prep.tile([128, n_chunks], F32, tag=f"comp{r}")
                    nc.sync.dma_start(
                        t_[:], lors_d[a][r, :].rearrange("(p c) -> p c", p=128))
                    comp.append(t_)
                P1X, P1Y, P2X, P2Y = comp
                PRJ = prep.tile([128, n_chunks], F32, tag="prj")
                nc.sync.dma_start(PRJ[:],
                                  proj_d[a][:].rearrange("(p c) -> p c", p=128))
                LNP = prep.tile([128, n_chunks], F32, tag="lnp")
                nc.scalar.activation(LNP[:], PRJ[:], AF.Ln)

                sides = []
                for (P1, P2, nm) in ((P1X, P2X, "x"), (P1Y, P2Y, "y")):
                    DX = prep.tile([128, n_chunks], F32, tag="dxt")
                    nc.vector.tensor_tensor(DX[:], P2[:], P1[:], op=AO.subtract)
                    CX = prep.tile([128, n_chunks, n_k], F32, tag="chainA")
                    tb = TT[:].unsqueeze(1).broadcast_to([128, n_chunks, n_k])
                    dxb = DX[:].unsqueeze(2).broadcast_to([128, n_chunks, n_k])
                    p1b = P1[:].unsqueeze(2).broadcast_to([128, n_chunks, n_k])
                    nc.vector.tensor_tensor(CX[:], tb, dxb, op=AO.mult)
                    nc.vector.tensor_tensor(CX[:], CX[:], p1b, op=AO.add)
                    Y_ = prep.tile([128, n_chunks, n_k], F32, tag="chainC")
                    nc.vector.tensor_scalar(Y_[:], CX[:], NEG_O, None, op0=AO.add)
                    Q_ = prep.tile([128, n_chunks, n_k], F32, tag="chainD")
                    nc.vector.tensor_scalar(Q_[:], Y_[:], INV_V, None, op0=AO.mult)
                    R_ = prep.tile([128, n_chunks, n_k], F32, tag="chainA")
                    nc.vector.tensor_tensor(R_[:], Y_[:], Q_[:], op=AO.subtract)
                    nc.vector.scalar_tensor_tensor(R_[:], Q_[:], -0.5, R_[:],
                                                   op0=AO.mult, op1=AO.add)
                    nc.vector.scalar_tensor_tensor(R_[:], Q_[:], -0.0625, R_[:],
                                                   op0=AO.mult, op1=AO.add)
                    U = prep.tile([128, n_chunks, n_k], F32, tag=f"u{nm}")
                    nc.vector.scalar_tensor_tensor(U[:], R_[:], INV_V, Q_[:],
                                                   op0=AO.mult, op1=AO.add)
                    nc.vector.tensor_scalar(U[:], U[:], 0.5, None,
                                            op0=AO.subtract)
                    IX0 = prep.tile([128, n_chunks, n_k], F32, tag="chainA")
                    nc.vector.tensor_scalar(IX0[:], U[:], MAGIC, MAGIC,
                                            op0=AO.add, op1=AO.subtract)
                    # window bounds, pre-scaled by -32 (exact in bf16):
                    # NLO = -32*(ix0+1) (max operand), NHI = -32*(ix0-1) (min)
                    NLO = prep.tile([128, n_chunks, n_k], F32, tag=f"nlo{nm}")
                    nc.vector.tensor_scalar(NLO[:], IX0[:], 1.0, -KCL,
                                            op0=AO.add, op1=AO.mult)
                    NHI = prep.tile([128, n_chunks, n_k], F32, tag=f"nhi{nm}")
                    nc.vector.tensor_scalar(NHI[:], IX0[:], 1.0, -KCL,
                                            op0=AO.subtract, op1=AO.mult)
                    sides.append((NLO, NHI, U))
                (NLOX, NHIX, UX), (NLOY, NHIY, UY) = sides

                PS = psp.tile([128, n_k, 128], F32, tag="ps")
                bank_slices = min(n_k, 4)

                half = n_k * 128
                for c in range(n_chunks):
                    first, last = c == 0, c == n_chunks - 1
                    # dense build: SQIN[:, si*half + k*128 : +128] = iota - u
                    # in-window, |.| >= 30 outside
                    SQIN = sqp.tile([128, 2 * half], F32, tag="sqin")
                    for si, (NLO, NHI, U) in enumerate(
                            ((NLOX, NHIX, UX), (NLOY, NHIY, UY))):
                        for k in range(n_k):
                            CL = workp.tile([128, 128], BF16, tag=f"cl{si}")
                            nc.vector.tensor_scalar(
                                CL[:], ION32[:], NLO[:, c, k:k + 1],
                                NHI[:, c, k:k + 1], op0=AO.max, op1=AO.min)
                            off = si * half + k * 128
                            nc.vector.scalar_tensor_tensor(
                                SQIN[:, off:off + 128], CL[:],
                                U[:, c, k:k + 1], JT33[:],
                                op0=AO.subtract, op1=AO.add)
                    SQT = sqp.tile([128, 2 * half], F32, tag="sqt")
                    nc.scalar.activation(SQT[:], SQIN[:], AF.Square,
                                         scale=SQRT_C)
                    WX = wp.tile([128, half], BF16, tag="wx")
                    nc.scalar.activation(WX[:], SQT[:, 0:half], AF.Exp,
                                         scale=-1.0)
                    WY = wp.tile([128, half], BF16, tag="wy")
                    nc.scalar.activation(WY[:], SQT[:, half:2 * half], AF.Exp,
                                         bias=LNP[:, c:c + 1], scale=-1.0)
                    for k in range(n_k):
                        nc.tensor.matmul(PS[:, k, :],
                                         WX[:, k * 128:(k + 1) * 128],
                                         WY[:, k * 128:(k + 1) * 128],
                                         start=first and (k % bank_slices == 0),
                                         stop=last and
                                         (k % bank_slices == bank_slices - 1))

                OUT = outp.tile([128, n_k, 128], F32, tag="out")
                nc.vector.tensor_copy(OUT[:], PS[:])
                nc.sync.dma_start(slab_d[a][:], OUT[:])
            if rep_ctx is not None:
                rep_ctx.__exit__(None, None, None)

    nc.finalize()
    return nc


def _host_tvals():
    zc = np.float32(-100.0) + (np.arange(128, dtype=np.float32)
                               + np.float32(0.5)) * np.float32(1.5625)
    return (zc + np.float32(100.0)) / np.float32(200.0)


def _host_prepare(inputs):
    iota = np.broadcast_to(np.arange(128, dtype=np.float32), (128, 128)).copy()
    t_all = _host_tvals()
    lors = {"x": inputs["xlors"], "y": inputs["ylors"], "z": inputs["zlors"]}
    proj = {"x": inputs["xproj"], "y": inputs["yproj"], "z": inputs["zproj"]}
    base = {}
    for ai, a in enumerate(AXES):
        cols = ROTATIONS[a] + [i + 3 for i in ROTATIONS[a]]
        l = np.asarray(lors[a]).astype(np.float32)[:, cols]
        base[f"lors{ai}"] = np.ascontiguousarray(
            np.stack([l[:, 0], l[:, 1], l[:, 3], l[:, 4]]))
        base[f"proj{ai}"] = np.ascontiguousarray(
            np.asarray(proj[a]), dtype=np.float32)
    in_maps = []
    for cid in range(N_CORES):
        m = dict(base)
        m["iota"] = iota
        tk = t_all[cid * N_K:(cid + 1) * N_K]
        m["tvals"] = np.broadcast_to(tk, (128, N_K)).copy()
        in_maps.append(m)
    return in_maps


def _host_gather(results):
    outs = []
    for ai, a in enumerate(AXES):
        bp = np.concatenate(
            [np.transpose(r[f"slab{ai}"], (0, 2, 1)) for r in results], axis=2)
        outs.append(np.ascontiguousarray(
            np.transpose(bp, BACK_ROTATIONS_IMAGE[a]).astype(np.float32)))
    return tuple(outs)


def kernel(image, xlors, ylors, zlors, xproj, yproj, zproj):
    from concourse.bass_utils import run_bass_kernel_spmd

    if "nc" not in _CACHE:
        _CACHE["nc"] = _build_kernel()
    nc = _CACHE["nc"]
    inputs = dict(xlors=np.asarray(xlors), ylors=np.asarray(ylors),
                  zlors=np.asarray(zlors), xproj=np.asarray(xproj),
                  yproj=np.asarray(yproj), zproj=np.asarray(zproj))
    in_maps = _host_prepare(inputs)
    res = run_bass_kernel_spmd(nc, in_maps, core_ids=list(range(N_CORES)))
    return _host_gather(res.results)


# revision 8
# speedup vs baseline: 2.6814x; 1.8346x over previous
"""PET tube-of-response backprojection on 8 TRN2 NeuronCores.

Strategy: slice-sharding. Every LOR crosses every slice of the dominant axis,
so giving core c slices [16c, 16c+16) of all three backprojections is
perfectly balanced, needs no collective, and each core's output is disjoint.

Per (axis, 128-LOR chunk) the scatter is computed as a dense separable outer
product on the tensor engine, with all 16 slices batched per instruction:
  VALL = iota - u*          (DVE tt, broadcast views, [128, 16, 128] per side)
  M    = |VALL| < 1.5       (DVE ts abs_max+is_lt -> 0/1 bf16, both sides)
  SQ   = Square(sqrt(c)*VALL)      (ACT, k-batched)
  E    = Exp(-SQ + 0.5*ln p)       (ACT, k-batched, bf16; both sides carry
                                    sqrt(p) so the outer product gives p)
  W    = E * M              (DVE tt bf16)
  PSUM[k] += Wx_k^T @ Wy_k  (PE, fp32 accumulation over chunks)

The window [ix0-1, ix0+1] (ix0 = round-to-nearest-even of u) is reproduced
EXACTLY by the strict test |i - u*| < 1.5 with u* = u + eps*(ix0 - u),
eps = 2^-10: writing u = ix0 + f (|f| <= 0.5), the test |i - ix0 - f(1-eps)|
< 1.5 admits exactly i in {ix0-1, ix0, ix0+1} since |f(1-eps)| < 0.5
strictly. Round-half-even ties (|f| = 0.5) land 0.5*eps ~ 5e-4 away from the
threshold - far above fp32 rounding noise. Weight perturbation from using
u* instead of u is <= 2c*1.5*0.5*eps ~ 2.6e-4 relative.

The voxel-index decision ix0 = round((cx+100)/1.5625 - 0.5) reproduces the
fp32 reference bit-exactly: cx via mult+add, the division via a
multiply + exact-residual correction (q = y*0.64; r = ((y-q)-0.5q)-0.0625q;
u' = q + r*0.64), and round-to-nearest-even via the +-1.5*2^23 magic add.

History: v1 (per-slice ACT ops, 7.4ms, ACT-bound) -> v2/v3 (k-batched ACT,
per-slice DVE clamp+marker pairs, 6.2ms: at the measured ~500ns/pair DVE
instruction floor) -> v4 (this): 4 batched DVE ops per chunk instead of 64.
"""

import math
import sys

sys.path.insert(0, "/opt/trn_rl_repo")
sys.path.insert(0, "/opt/trn_rl_repo/concourse")

import numpy as np

V = 1.5625
INV_V = float(np.float32(0.64))
NEG_O = 100.0
SIGMA2 = 9.0 * math.pi / 4.0
C = 0.5 * V * V / SIGMA2
SQRT_C = math.sqrt(C)
MAGIC = 12582912.0
EPS_BLEND = 1.0 / 1024.0

N_CORES = 8
N_K = 16          # slices per core
N_CHUNKS = 128    # 128-LOR chunks
N_LORS = N_CHUNKS * 128

ROTATIONS = {"x": [1, 2, 0], "y": [0, 2, 1], "z": [0, 1, 2]}
BACK_ROTATIONS_IMAGE = {"x": [1, 2, 0], "y": [1, 0, 2], "z": [0, 1, 2]}
AXES = ("x", "y", "z")

_CACHE = {}


def _build_kernel(repeat=1):
    from concourse import mybir, tile, bacc

    DT = mybir.dt
    F32 = DT.float32
    BF16 = DT.bfloat16
    AO = mybir.AluOpType
    AF = mybir.ActivationFunctionType
    n_chunks, n_k, n_axes = N_CHUNKS, N_K, 3

    nc = bacc.Bacc("TRN2", target_bir_lowering=False, debug=False)
    lors_d = [nc.dram_tensor(f"lors{a}", [4, N_LORS], F32, kind="ExternalInput")
              for a in range(n_axes)]
    proj_d = [nc.dram_tensor(f"proj{a}", [N_LORS], F32, kind="ExternalInput")
              for a in range(n_axes)]
    iota_d = nc.dram_tensor("iota", [128, 128], F32, kind="ExternalInput")
    tval_d = nc.dram_tensor("tvals", [128, n_k], F32, kind="ExternalInput")
    slab_d = [nc.dram_tensor(f"slab{a}", [128, n_k, 128], F32,
                             kind="ExternalOutput") for a in range(n_axes)]

    with tile.TileContext(nc) as tc:
        with (
            tc.tile_pool(name="const", bufs=1) as constp,
            tc.tile_pool(name="pre", bufs=1) as prep,
            tc.tile_pool(name="val", bufs=2) as valp,
            tc.tile_pool(name="sqt", bufs=1) as sqtp,
            tc.tile_pool(name="msk", bufs=1) as mskp,
            tc.tile_pool(name="e", bufs=2) as ep,
            tc.tile_pool(name="w", bufs=2) as wp,
            tc.tile_pool(name="out", bufs=2) as outp,
            tc.tile_pool(name="ps", bufs=2, space="PSUM") as psp,
        ):
            IOTA = constp.tile([128, 128], F32, tag="iota")
            nc.sync.dma_start(IOTA[:], iota_d[:])
            TT = constp.tile([128, n_k], F32, tag="tt")
            nc.sync.dma_start(TT[:], tval_d[:])

            rep_ctx = tc.For_i(0, repeat, 1) if repeat > 1 else None
            if rep_ctx is not None:
                rep_ctx.__enter__()
            for a in range(n_axes):
                comp = []
                for r in range(4):
                    t_ = prep.tile([128, n_chunks], F32, tag=f"comp{r}")
                    nc.sync.dma_start(
                        t_[:], lors_d[a][r, :].rearrange("(p c) -> p c", p=128))
                    comp.append(t_)
                P1X, P1Y, P2X, P2Y = comp
                PRJ = prep.tile([128, n_chunks], F32, tag="prj")
                nc.sync.dma_start(PRJ[:],
                                  proj_d[a][:].rearrange("(p c) -> p c", p=128))
                LNP = prep.tile([128, n_chunks], F32, tag="lnp")
                nc.scalar.activation(LNP[:], PRJ[:], AF.Ln)
                LNPH = prep.tile([128, n_chunks], F32, tag="lnph")
                nc.vector.tensor_scalar(LNPH[:], LNP[:], 0.5, None,
                                        op0=AO.mult)

                sides = []
                for (P1, P2, nm) in ((P1X, P2X, "x"), (P1Y, P2Y, "y")):
                    DX = prep.tile([128, n_chunks], F32, tag="dxt")
                    nc.vector.tensor_tensor(DX[:], P2[:], P1[:], op=AO.subtract)
                    CX = prep.tile([128, n_chunks, n_k], F32, tag="chainA")
                    tb = TT[:].unsqueeze(1).broadcast_to([128, n_chunks, n_k])
                    dxb = DX[:].unsqueeze(2).broadcast_to([128, n_chunks, n_k])
                    p1b = P1[:].unsqueeze(2).broadcast_to([128, n_chunks, n_k])
                    nc.vector.tensor_tensor(CX[:], tb, dxb, op=AO.mult)
                    nc.vector.tensor_tensor(CX[:], CX[:], p1b, op=AO.add)
                    Y_ = prep.tile([128, n_chunks, n_k], F32, tag="chainC")
                    nc.vector.tensor_scalar(Y_[:], CX[:], NEG_O, None, op0=AO.add)
                    Q_ = prep.tile([128, n_chunks, n_k], F32, tag="chainD")
                    nc.vector.tensor_scalar(Q_[:], Y_[:], INV_V, None, op0=AO.mult)
                    R_ = prep.tile([128, n_chunks, n_k], F32, tag="chainA")
                    nc.vector.tensor_tensor(R_[:], Y_[:], Q_[:], op=AO.subtract)
                    nc.vector.scalar_tensor_tensor(R_[:], Q_[:], -0.5, R_[:],
                                                   op0=AO.mult, op1=AO.add)
                    nc.vector.scalar_tensor_tensor(R_[:], Q_[:], -0.0625, R_[:],
                                                   op0=AO.mult, op1=AO.add)
                    U = prep.tile([128, n_chunks, n_k], F32, tag=f"u{nm}")
                    nc.vector.scalar_tensor_tensor(U[:], R_[:], INV_V, Q_[:],
                                                   op0=AO.mult, op1=AO.add)
                    nc.vector.tensor_scalar(U[:], U[:], 0.5, None,
                                            op0=AO.subtract)
                    IX0 = prep.tile([128, n_chunks, n_k], F32, tag="chainA")
                    nc.vector.tensor_scalar(IX0[:], U[:], MAGIC, MAGIC,
                                            op0=AO.add, op1=AO.subtract)
                    # u* = (1-eps)*u + eps*ix0: tie-exact window blend
                    UB = prep.tile([128, n_chunks, n_k], F32, tag=f"ub{nm}")
                    nc.vector.tensor_scalar(UB[:], U[:], 1.0 - EPS_BLEND, None,
                                            op0=AO.mult)
                    nc.vector.scalar_tensor_tensor(UB[:], IX0[:], EPS_BLEND,
                                                   UB[:], op0=AO.mult,
                                                   op1=AO.add)
                    sides.append(UB)
                UBX, UBY = sides

                PS = psp.tile([128, n_k, 128], F32, tag="ps")
                bank_slices = min(n_k, 4)
                iota_b = IOTA[:].unsqueeze(1).broadcast_to([128, n_k, 128])

                for c in range(n_chunks):
                    first, last = c == 0, c == n_chunks - 1
                    VALL = valp.tile([128, 2, n_k, 128], F32, tag="vall")
                    for si, UB in enumerate((UBX, UBY)):
                        ub_b = UB[:, c, :].unsqueeze(2).broadcast_to(
                            [128, n_k, 128])
                        nc.vector.tensor_tensor(VALL[:, si], iota_b, ub_b,
                                                op=AO.subtract)
                    SQT = sqtp.tile([128, 2, n_k, 128], F32, tag="sqt")
                    nc.scalar.activation(SQT[:], VALL[:], AF.Square,
                                         scale=SQRT_C)
                    # in-window iff (sqrt(c)*(i-u*))^2 < c*2.25; margin from
                    # the u* blend is ~6e-4 relative, far above fp32 noise
                    M16 = mskp.tile([128, 2, n_k, 128], BF16, tag="m16")
                    nc.vector.tensor_scalar(M16[:], SQT[:], C * 2.25, None,
                                            op0=AO.is_lt)
                    E16 = ep.tile([128, 2, n_k, 128], BF16, tag="e16")
                    nc.scalar.activation(E16[:], SQT[:], AF.Exp,
                                         bias=LNPH[:, c:c + 1], scale=-1.0)
                    W16 = wp.tile([128, 2, n_k, 128], BF16, tag="w16")
                    nc.vector.tensor_tensor(W16[:], E16[:], M16[:], op=AO.mult)
                    for k in range(n_k):
                        nc.tensor.matmul(PS[:, k, :], W16[:, 0, k, :],
                                         W16[:, 1, k, :],
                                         start=first and (k % bank_slices == 0),
                                         stop=last and
                                         (k % bank_slices == bank_slices - 1))

                OUT = outp.tile([128, n_k, 128], F32, tag="out")
                nc.vector.tensor_copy(OUT[:], PS[:])
                nc.sync.dma_start(slab_d[a][:], OUT[:])
            if rep_ctx is not None:
                rep_ctx.__exit__(None, None, None)

    nc.finalize()
    return nc


def _host_tvals():
    zc = np.float32(-100.0) + (np.arange(128, dtype=np.float32)
                               + np.float32(0.5)) * np.float32(1.5625)
    return (zc + np.float32(100.0)) / np.float32(200.0)


def _host_prepare(inputs):
    iota = np.broadcast_to(np.arange(128, dtype=np.float32), (128, 128)).copy()
    t_all = _host_tvals()
    lors = {"x": inputs["xlors"], "y": inputs["ylors"], "z": inputs["zlors"]}
    proj = {"x": inputs["xproj"], "y": inputs["yproj"], "z": inputs["zproj"]}
    base = {}
    for ai, a in enumerate(AXES):
        cols = ROTATIONS[a] + [i + 3 for i in ROTATIONS[a]]
        l = np.asarray(lors[a]).astype(np.float32)[:, cols]
        base[f"lors{ai}"] = np.ascontiguousarray(
            np.stack([l[:, 0], l[:, 1], l[:, 3], l[:, 4]]))
        base[f"proj{ai}"] = np.ascontiguousarray(
            np.asarray(proj[a]), dtype=np.float32)
    in_maps = []
    for cid in range(N_CORES):
        m = dict(base)
        m["iota"] = iota
        tk = t_all[cid * N_K:(cid + 1) * N_K]
        m["tvals"] = np.broadcast_to(tk, (128, N_K)).copy()
        in_maps.append(m)
    return in_maps


def _host_gather(results):
    outs = []
    for ai, a in enumerate(AXES):
        bp = np.concatenate(
            [np.transpose(r[f"slab{ai}"], (0, 2, 1)) for r in results], axis=2)
        outs.append(np.ascontiguousarray(
            np.transpose(bp, BACK_ROTATIONS_IMAGE[a]).astype(np.float32)))
    return tuple(outs)


def kernel(image, xlors, ylors, zlors, xproj, yproj, zproj):
    from concourse.bass_utils import run_bass_kernel_spmd

    if "nc" not in _CACHE:
        _CACHE["nc"] = _build_kernel()
    nc = _CACHE["nc"]
    inputs = dict(xlors=np.asarray(xlors), ylors=np.asarray(ylors),
                  zlors=np.asarray(zlors), xproj=np.asarray(xproj),
                  yproj=np.asarray(yproj), zproj=np.asarray(zproj))
    in_maps = _host_prepare(inputs)
    res = run_bass_kernel_spmd(nc, in_maps, core_ids=list(range(N_CORES)))
    return _host_gather(res.results)
